# revision 1
# baseline (speedup 1.0000x reference)
"""GCN link predictor on 8 trn2 NeuronCores (Bass/Tile SPMD).

Algorithm (per core c of 8):
  nodes sharded contiguously (12500/core); edges (+self-loops) partitioned by
  dst owner, sorted by dst, padded to a uniform K tiles of 128 per 128-node
  dst block (identical instruction stream on every core; per-core data only).

  repeat for layers (W1,relu) (W2,relu) (W3,-):
    dense:  hhat_slice = (z_slice @ W) * dinv[row]       (PE transpose + mm)
    AG:     AllGather hhat slices -> hhat_full [N, C]
    agg:    per dst block: acc(psum) = sum_k S_k^T msg_k   (S from iota==dstloc)
            z_blk = relu?(acc * dinv_blk + bias)
  AG z3 -> z3_full; decode pos/neg: per 128-edge tile gather z3[src], z3[dst],
  fused mul+reduce -> score column; PE-transpose score tiles -> flat scores.

Self-contained: hardcodes all shapes for the nn_GCNLinkPredictor problem.
"""
import numpy as np

import concourse.bacc as bacc
import concourse.bass as bass
import concourse.mybir as mybir
import concourse.tile as tile
from concourse.bass_utils import run_bass_kernel_spmd
from concourse.masks import make_identity

P = 128
N = 100000
E = 1600000
M = 8
NPC = N // M                 # 12500
BPC = (NPC + P - 1) // P     # 98
SLICE = BPC * P              # 12544
CIN = 128
CH = 128
COUT = 64
EPC = E // M                 # 200000
TD = (EPC + P - 1) // P      # 1563
DG = (TD + P - 1) // P       # 13 decode transpose groups


PHASES = None     # None = all; else set of phase names to include
DECODE_TTR = False  # fused tensor_tensor_reduce breaks the HW compile path


def _configure(n, e):
    """Switch module to a smaller problem size (for fast validation)."""
    global N, E, NPC, BPC, SLICE, EPC, TD, DG
    N, E = n, e
    NPC = N // M
    BPC = (NPC + P - 1) // P
    SLICE = BPC * P
    EPC = E // M
    TD = (EPC + P - 1) // P
    DG = (TD + P - 1) // P

f32 = mybir.dt.float32
i32 = mybir.dt.int32


# --------------------------- host preprocessing ---------------------------

def _prep_agg(pos_edge_index):
    src = np.concatenate([pos_edge_index[0], np.arange(N, dtype=np.int64)])
    dst = np.concatenate([pos_edge_index[1], np.arange(N, dtype=np.int64)])
    deg = np.bincount(dst, minlength=N).astype(np.float32)
    dinv = np.where(deg > 0, 1.0 / np.sqrt(deg), 0.0).astype(np.float32)

    order = np.argsort(dst, kind="stable")
    src_s, dst_s = src[order], dst[order]
    core_of = dst_s // NPC
    blk = (dst_s - core_of * NPC) // P
    counts = np.zeros((M, BPC), dtype=np.int64)
    np.add.at(counts, (core_of, blk), 1)
    K = int(np.max((counts + P - 1) // P))
    T = BPC * K

    cores = []
    for c in range(M):
        sel = core_of == c
        s_c, d_c = src_s[sel], dst_s[sel]
        b_c = (d_c - c * NPC) // P
        srcf = np.full(T * P, c * NPC, dtype=np.int64)
        dstloc = np.full(T * P, -1.0, dtype=np.float32)
        start = np.searchsorted(b_c, np.arange(BPC))
        end = np.searchsorted(b_c, np.arange(BPC) + 1)
        for b in range(BPC):
            n_b = end[b] - start[b]
            base = b * K * P
            srcf[base:base + n_b] = s_c[start[b]:end[b]]
            dstloc[base:base + n_b] = (d_c[start[b]:end[b]] -
                                       (c * NPC + b * P)).astype(np.float32)
        dinvT = np.zeros((P, BPC), dtype=np.float32)
        for b in range(BPC):
            lo = c * NPC + b * P
            hi = min(lo + P, (c + 1) * NPC)
            dinvT[:hi - lo, b] = dinv[lo:hi]
        cores.append(dict(
            srcT=np.ascontiguousarray(srcf.reshape(T, P).T.astype(np.int32)),
            dstlocT=np.ascontiguousarray(dstloc.reshape(T, P).T),
            dinvT=dinvT))
    return K, T, dinv, cores


def _prep_decode(edge_index):
    cores = []
    for c in range(M):
        s = edge_index[0, c * EPC:(c + 1) * EPC].astype(np.int32)
        d = edge_index[1, c * EPC:(c + 1) * EPC].astype(np.int32)
        pad = TD * P - EPC
        s = np.concatenate([s, np.zeros(pad, np.int32)])
        d = np.concatenate([d, np.zeros(pad, np.int32)])
        cores.append((np.ascontiguousarray(s.reshape(TD, P).T),
                      np.ascontiguousarray(d.reshape(TD, P).T)))
    return cores


# ----------------------------- device builder -----------------------------

def build_nc(K, reps=1):
    T = BPC * K
    nc = bacc.Bacc(None, target_bir_lowering=False)
    with tile.TileContext(nc) as tc:
        with tc.tile_pool(name="dram", bufs=1, space="DRAM") as dram, \
             tc.tile_pool(name="cst", bufs=1) as cst, \
             tc.tile_pool(name="xt", bufs=3) as xtp, \
             tc.tile_pool(name="sS", bufs=6) as sSp, \
             tc.tile_pool(name="msg", bufs=12) as msgp, \
             tc.tile_pool(name="zb", bufs=4) as zbp, \
             tc.tile_pool(name="ps", bufs=2, space="PSUM") as psp, \
             tc.tile_pool(name="acc", bufs=2, space="PSUM") as accp:

            # ---------------- I/O ----------------
            def ein(name, shape, dtype=f32):
                return dram.tile(shape, dtype, kind="ExternalInput",
                                 name=name, uniquify=False)

            x_s = ein("x_s", [SLICE, CIN])
            W1 = ein("W1", [CIN, CH]); W2 = ein("W2", [CH, CH])
            W3 = ein("W3", [CH, COUT])
            bb1 = ein("bb1", [P, CH]); bb2 = ein("bb2", [P, CH])
            bb3 = ein("bb3", [P, COUT])
            dinvT = ein("dinvT", [P, BPC])
            srcT = ein("srcT", [P, T], i32)
            dstlocT = ein("dstlocT", [P, T])
            ps_idx = ein("ps_idx", [P, TD], i32)
            pd_idx = ein("pd_idx", [P, TD], i32)
            ns_idx = ein("ns_idx", [P, TD], i32)
            nd_idx = ein("nd_idx", [P, TD], i32)

            pos_out = dram.tile([TD, P], f32, kind="ExternalOutput",
                                name="pos_out", uniquify=False)
            neg_out = dram.tile([TD, P], f32, kind="ExternalOutput",
                                name="neg_out", uniquify=False)

            # internal DRAM
            hs1 = dram.tile([SLICE, CH], f32, name="hs1")
            hs2 = dram.tile([SLICE, CH], f32, name="hs2")
            hs3 = dram.tile([SLICE, COUT], f32, name="hs3")
            z1s = dram.tile([SLICE, CH], f32, name="z1s")
            z2s = dram.tile([SLICE, CH], f32, name="z2s")
            z3s = dram.tile([SLICE, COUT], f32, name="z3s")
            hf1 = dram.tile([N, CH], f32, name="hf1", addr_space="Shared")
            hf2 = dram.tile([N, CH], f32, name="hf2", addr_space="Shared")
            hf3 = dram.tile([N, COUT], f32, name="hf3", addr_space="Shared")
            z3f = dram.tile([N, COUT], f32, name="z3f", addr_space="Shared")

            # ---------------- constants to SBUF ----------------
            W1_sb = cst.tile([CIN, CH], f32)
            W2_sb = cst.tile([CH, CH], f32)
            W3_sb = cst.tile([CH, COUT], f32)
            bb1_sb = cst.tile([P, CH], f32)
            bb2_sb = cst.tile([P, CH], f32)
            bb3_sb = cst.tile([P, COUT], f32)
            dinv_sb = cst.tile([P, BPC], f32)
            srcT_sb = cst.tile([P, T], i32)
            dstloc_sb = cst.tile([P, T], f32)
            pidx_sb = [cst.tile([P, TD], i32, name=f"pidx{i}") for i in range(2)]
            nidx_sb = [cst.tile([P, TD], i32, name=f"nidx{i}") for i in range(2)]
            for dst_t, src_t in [(W1_sb, W1), (W2_sb, W2), (W3_sb, W3),
                                 (bb1_sb, bb1), (bb2_sb, bb2), (bb3_sb, bb3),
                                 (dinv_sb, dinvT), (srcT_sb, srcT),
                                 (dstloc_sb, dstlocT),
                                 (pidx_sb[0], ps_idx), (pidx_sb[1], pd_idx),
                                 (nidx_sb[0], ns_idx), (nidx_sb[1], nd_idx)]:
                nc.sync.dma_start(out=dst_t[:], in_=src_t[:])

            ident = cst.tile([P, P], f32)
            make_identity(nc, ident[:])
            iota_i = cst.tile([P, P], i32)
            nc.gpsimd.iota(iota_i[:], pattern=[[1, P]], base=0,
                           channel_multiplier=0)
            iota_f = cst.tile([P, P], f32)
            nc.vector.tensor_copy(out=iota_f[:], in_=iota_i[:])

            # ---------------- phases ----------------
            def dense(z_in, W_sb, cout, hs_out, scope):
                with nc.named_scope(scope):
                    for i in range(BPC):
                        zt = xtp.tile([P, CH], f32, tag="zt")
                        nc.sync.dma_start(out=zt[:],
                                          in_=z_in[i * P:(i + 1) * P, :])
                        tp = psp.tile([P, CH], f32, tag="tp")
                        nc.tensor.transpose(out=tp[:], in_=zt[:],
                                            identity=ident[:])
                        zT = xtp.tile([P, CH], f32, tag="zT")
                        nc.vector.tensor_copy(out=zT[:], in_=tp[:])
                        hp = psp.tile([P, cout], f32, tag="hp")
                        nc.tensor.matmul(out=hp[:], lhsT=zT[:],
                                         rhs=W_sb[:, :cout],
                                         start=True, stop=True)
                        hh = zbp.tile([P, cout], f32, tag="hh")
                        nc.vector.tensor_scalar(
                            out=hh[:], in0=hp[:],
                            scalar1=dinv_sb[:, i:i + 1], scalar2=None,
                            op0=mybir.AluOpType.mult)
                        nc.sync.dma_start(
                            out=hs_out[i * P:(i + 1) * P, :], in_=hh[:])

            def allgather(slice_t, full_t, scope):
                with nc.named_scope(scope):
                    nc.gpsimd.collective_compute(
                        "AllGather", mybir.AluOpType.bypass,
                        replica_groups=[list(range(M))],
                        ins=[slice_t[:NPC, :]],
                        outs=[full_t[:]])

            def agg(hf, cout, bias_sb, relu, z_out, scope):
                with nc.named_scope(scope):
                    for b in range(BPC):
                        acc = accp.tile([P, cout], f32, tag="acc")
                        for k in range(K):
                            t = b * K + k
                            S = sSp.tile([P, P], f32, tag="S")
                            nc.vector.tensor_tensor(
                                out=S[:],
                                in0=dstloc_sb[:, t:t + 1].to_broadcast([P, P]),
                                in1=iota_f[:],
                                op=mybir.AluOpType.is_equal)
                            msg = msgp.tile([P, cout], f32, tag="msg")
                            nc.gpsimd.indirect_dma_start(
                                out=msg[:], out_offset=None,
                                in_=hf[:],
                                in_offset=bass.IndirectOffsetOnAxis(
                                    ap=srcT_sb[:, t:t + 1], axis=0))
                            nc.tensor.matmul(out=acc[:], lhsT=S[:],
                                             rhs=msg[:],
                                             start=(k == 0), stop=(k == K - 1))
                        zb = zbp.tile([P, cout], f32, tag="zb")
                        nc.vector.tensor_scalar(
                            out=zb[:], in0=acc[:],
                            scalar1=dinv_sb[:, b:b + 1], scalar2=None,
                            op0=mybir.AluOpType.mult)
                        nc.vector.tensor_tensor(out=zb[:], in0=zb[:],
                                                in1=bias_sb[:, :cout],
                                                op=mybir.AluOpType.add)
                        if relu:
                            nc.vector.tensor_scalar_max(zb[:], zb[:], 0.0)
                        nc.sync.dma_start(out=z_out[b * P:(b + 1) * P, :],
                                          in_=zb[:])

            def decode(idx_pair, out_t, scope):
                with nc.named_scope(scope):
                    for g in range(DG):
                        ntile = min(P, TD - g * P)
                        sc = sSp.tile([P, P], f32, tag="sc")
                        if ntile < P:
                            nc.vector.memset(sc[:], 0.0)
                        for tt in range(ntile):
                            t = g * P + tt
                            za = msgp.tile([P, COUT], f32, tag="za")
                            zbt = msgp.tile([P, COUT], f32, tag="zbt")
                            nc.gpsimd.indirect_dma_start(
                                out=za[:], out_offset=None, in_=z3f[:],
                                in_offset=bass.IndirectOffsetOnAxis(
                                    ap=idx_pair[0][:, t:t + 1], axis=0))
                            nc.gpsimd.indirect_dma_start(
                                out=zbt[:], out_offset=None, in_=z3f[:],
                                in_offset=bass.IndirectOffsetOnAxis(
                                    ap=idx_pair[1][:, t:t + 1], axis=0))
                            prod = zbp.tile([P, COUT], f32, tag="prod")
                            if DECODE_TTR:
                                nc.vector.tensor_tensor_reduce(
                                    out=prod[:], in0=za[:], in1=zbt[:],
                                    scale=1.0, scalar=0.0,
                                    op0=mybir.AluOpType.mult,
                                    op1=mybir.AluOpType.add,
                                    accum_out=sc[:, tt:tt + 1])
                            else:
                                nc.vector.tensor_tensor(
                                    out=prod[:], in0=za[:], in1=zbt[:],
                                    op=mybir.AluOpType.mult)
                                nc.vector.tensor_reduce(
                                    out=sc[:, tt:tt + 1], in_=prod[:],
                                    axis=mybir.AxisListType.X,
                                    op=mybir.AluOpType.add)
                        tp = psp.tile([P, P], f32, tag="tp")
                        nc.tensor.transpose(out=tp[:], in_=sc[:],
                                            identity=ident[:])
                        so = zbp.tile([P, P], f32, tag="so")
                        nc.vector.tensor_copy(out=so[:], in_=tp[:])
                        nc.sync.dma_start(
                            out=out_t[g * P:g * P + ntile, :],
                            in_=so[:ntile, :])

            def on(p):
                return PHASES is None or p in PHASES

            def run_pipeline():
                if on("dense1"):
                    dense(x_s, W1_sb, CH, hs1, "dense1")
                if on("ag1"):
                    allgather(hs1, hf1, "ag1")
                if on("agg1"):
                    agg(hf1, CH, bb1_sb, True, z1s, "agg1")
                if on("dense2"):
                    dense(z1s, W2_sb, CH, hs2, "dense2")
                if on("ag2"):
                    allgather(hs2, hf2, "ag2")
                if on("agg2"):
                    agg(hf2, CH, bb2_sb, True, z2s, "agg2")
                if on("dense3"):
                    dense(z2s, W3_sb, COUT, hs3, "dense3")
                if on("ag3"):
                    allgather(hs3, hf3, "ag3")
                if on("agg3"):
                    agg(hf3, COUT, bb3_sb, False, z3s, "agg3")
                if on("ag4"):
                    allgather(z3s, z3f, "ag4")
                if on("dec"):
                    decode(pidx_sb, pos_out, "dec_pos")
                    decode(nidx_sb, neg_out, "dec_neg")

            if reps > 1:
                with tc.For_i(0, reps, 1):
                    run_pipeline()
            else:
                run_pipeline()
    nc.compile()
    return nc


_CACHE = {}


def _make_in_maps(x, W1, b1, W2, b2, W3, b3, pe, ne):
    K, T, dinv, agg_cores = _prep_agg(pe)
    dec_pos = _prep_decode(pe)
    dec_neg = _prep_decode(ne)
    in_maps = []
    for c in range(M):
        xs = np.zeros((SLICE, CIN), np.float32)
        xs[:NPC] = x[c * NPC:(c + 1) * NPC]
        a = agg_cores[c]
        in_maps.append({
            "x_s": xs, "W1": W1, "W2": W2, "W3": W3,
            "bb1": np.tile(b1[None, :], (P, 1)).astype(np.float32),
            "bb2": np.tile(b2[None, :], (P, 1)).astype(np.float32),
            "bb3": np.tile(b3[None, :], (P, 1)).astype(np.float32),
            "dinvT": a["dinvT"], "srcT": a["srcT"], "dstlocT": a["dstlocT"],
            "ps_idx": dec_pos[c][0], "pd_idx": dec_pos[c][1],
            "ns_idx": dec_neg[c][0], "nd_idx": dec_neg[c][1],
        })
    return K, in_maps


def _run(in_maps, K, reps=1):
    key = (K, reps)
    if key not in _CACHE:
        _CACHE[key] = build_nc(K, reps=reps)
    res = run_bass_kernel_spmd(_CACHE[key], in_maps,
                               core_ids=list(range(M)))
    pos = np.concatenate(
        [res.results[c]["pos_out"].ravel()[:EPC] for c in range(M)])
    neg = np.concatenate(
        [res.results[c]["neg_out"].ravel()[:EPC] for c in range(M)])
    return pos, neg


def kernel(x, W1, b1, W2, b2, W3, b3, pos_edge_index, neg_edge_index):
    x = np.asarray(x, dtype=np.float32)
    W1 = np.asarray(W1, np.float32); b1 = np.asarray(b1, np.float32)
    W2 = np.asarray(W2, np.float32); b2 = np.asarray(b2, np.float32)
    W3 = np.asarray(W3, np.float32); b3 = np.asarray(b3, np.float32)
    pe = np.asarray(pos_edge_index).astype(np.int64)
    ne = np.asarray(neg_edge_index).astype(np.int64)
    K, in_maps = _make_in_maps(x, W1, b1, W2, b2, W3, b3, pe, ne)
    return _run(in_maps, K, reps=1)

